# revision 5
# baseline (speedup 1.0000x reference)
"""Trainium2 Bass kernel for the torch-faithful MultiHeadAttention module.

Math (validated vs the jax reference):
  qkv = x @ W_qkv.T + b_qkv                    # [B, S, 3E]
  qkv.view(B, H, -1, 3*hd)  is a PLAIN reshape, so "head" h is really the
  sequence block s in [128h, 128h+128), and within a head the 2048 rows are
  s' = (s%128)*16 + j with j = f//192; q/k/v are column slices of each
  192-wide block j.
  score = q @ k.T / 8 ; softmax ; context ; out = context' @ W_out.T + b_out

Sharding (8 cores): data-parallel over batch (4 cores per batch element),
head-parallel within the group (4 heads per core).  Each core computes its
heads' attention entirely on-chip (flash style, no HBM score matrix) and a
partial out-projection over its 256 context columns; the host sums the 4
partials per batch element (a pure unshard/reduce step) and adds b_out.

Internally each head uses the s'' = j*128 + r ordering (a permutation of
s'); the permutation is undone for free in the final strided DMA to DRAM.

Performance structure (vs the first-generation kernel):
- Scores run as ROW-TILED pairs: the K=64 contraction only fills half the
  PE array, so two kt-tiles (one odd-j with K at partitions 0-63, one
  even-j at 64-127) execute concurrently via tile_position=(0,0)/(64,0).
  The block structure of W_qkv^T makes the k parity split land on the
  right partition halves for free; q is written to both halves.
- Softmax normalization: the ones-column of v_aug gives the row sums l as
  PSUM row 64; 1/l comes from a single DVE reciprocal on that row, is
  broadcast across partitions by an SBUF->SBUF DMA, and one fused DVE
  multiply produces normalized ctx^T directly from PSUM.  No DRAM bounce,
  no PE transposes at chunk boundaries.
- Chunk-major (c, head) order; the out-projection of the first q-chunk is
  interleaved into the second chunk's attention; output partials are bf16.
"""

import numpy as np

import concourse.bass as bass
import concourse.mybir as mybir
import concourse.tile as tile
from concourse import bacc
from concourse.bass_utils import run_bass_kernel_spmd
from concourse.masks import make_identity

B, S, E = 2, 2048, 1024
H, HD = 16, 64
NH = 4   # heads per core
NJ = 16  # 192-wide column blocks in 3E == kt tiles per head
P = 128
ET = E // P  # 8 contraction tiles of 128
CH = 1024    # q-chunk width
F32 = mybir.dt.float32
BF16 = mybir.dt.bfloat16
EXP = mybir.ActivationFunctionType.Exp
IDENT = mybir.ActivationFunctionType.Identity

_NC_CACHE = None
_LAST_RESULT = None  # BassKernelResults of the most recent run (for test harness)


def _emit(nc, tc, xT, wqkvT, woutT, bblk, outp):
    import contextlib

    with contextlib.ExitStack() as ctx:
        ctx.enter_context(
            nc.allow_low_precision(reason="bf16 matmul operands")
        )
        const = ctx.enter_context(tc.tile_pool(name="const", bufs=1))
        vtmp = ctx.enter_context(tc.tile_pool(name="vtmp", bufs=2))
        ppool = ctx.enter_context(tc.tile_pool(name="probs", bufs=4))
        rpool = ctx.enter_context(tc.tile_pool(name="recip", bufs=2))
        opool = ctx.enter_context(tc.tile_pool(name="ostage", bufs=2))
        # PSUM: "slot" ring 2 x [128,2,512]f32 (4 banks) shared by scores /
        # proj / v-transposes / outproj, + "ctx" ring 2 x [65,1024]f32
        # (4 banks).  Total exactly 8 banks.
        ps = ctx.enter_context(tc.tile_pool(name="ps", bufs=2, space="PSUM"))

        # ---- resident tiles -------------------------------------------------
        xT_sb = const.tile([P, ET, NH * P], BF16, tag="xT")  # [128, 8, 512]
        for et in range(ET):
            nc.sync.dma_start(out=xT_sb[:, et, :], in_=xT[et, :, :])

        bblk_sb = const.tile([P, 24], F32, tag="bblk")
        nc.sync.dma_start(out=bblk_sb, in_=bblk[:, :])

        # W_qkv^T fully resident in bf16: [128, 8, 3072] = 48KB/partition.
        # One DMA per 128-wide column block, in consumption order.
        wq_all = const.tile([P, ET, 3 * E], BF16, tag="wq")
        for b in range(24):
            for et in range(ET):
                nc.sync.dma_start(
                    out=wq_all[:, et, b * P:(b + 1) * P],
                    in_=wqkvT[et, :, b * P:(b + 1) * P],
                )

        woutT_sb = const.tile([P, 2, E], BF16, tag="woutT")  # [128, 2, 1024]
        for t in range(2):
            nc.gpsimd.dma_start(out=woutT_sb[:, t, :], in_=woutT[t, :, :])

        ident = const.tile([P, P], BF16, tag="ident")
        make_identity(nc, ident)

        # q^T, replicated on both partition halves: [128, nh, nj, 128]
        qT = const.tile([P, NH, NJ, P], BF16, tag="qT")
        # k^T parity-split: partitions 0-63 hold k_{2t+1}, 64-127 hold k_{2t}
        kT = const.tile([P, NH, NJ // 2, P], BF16, tag="kT")
        # v_aug per head per j-block: [128 rows, 64 v cols + 1 ones col]
        vaug = const.tile([P, NH, NJ, HD + 1], BF16, tag="vaug")
        nc.vector.memset(vaug[:, :, :, HD:HD + 1], 1.0)
        # normalized context^T: K-tile t holds heads (2t, 2t+1) on halves
        ctxT = const.tile([P, 2, S], BF16, tag="ctxT")

        # ---- qkv projection helpers ----------------------------------------
        # Block pattern per two j:
        #   b=3m: [q_2m | k_2m]  b=3m+1: [v_2m | q_2m+1]  b=3m+2: [k_2m+1 | v_2m+1]
        # ps rows 0-63 are the first half (q/v/k resp.), 64-127 the second.
        vts = {}

        def copy(ps_b, rows, bcol, out_ap, eng):
            src = ps_b[rows[0]:rows[1], :]
            if out_ap.shape[-1] == P and len(out_ap.shape) > 2:
                src = src.rearrange("d (nh p) -> d nh p", p=P)
            bias = bblk_sb[rows[0]:rows[1], bcol:bcol + 1]
            if eng == "act":
                nc.scalar.activation(out=out_ap, in_=src, func=IDENT, bias=bias)
            else:
                nc.vector.tensor_scalar_add(out=out_ap, in0=src, scalar1=bias)

        def finish_v(j, eng):
            vt_j = vts.pop(j)
            ps_tr = ps.tile([P, NH, HD], BF16, tag="slot")
            for i in range(NH):
                nc.tensor.transpose(
                    ps_tr[:, i, :], vt_j[:, i * P:(i + 1) * P],
                    ident[0:HD, 0:HD],
                )
            if eng == "act":
                nc.scalar.copy(out=vaug[:, :, j, 0:HD], in_=ps_tr)
            else:
                nc.vector.tensor_copy(out=vaug[:, :, j, 0:HD], in_=ps_tr)

        def proj_block(b, eng):
            ps_b = ps.tile([P, NH * P], F32, tag="slot")
            for et in range(ET):
                nc.tensor.matmul(
                    ps_b,
                    lhsT=wq_all[:, et, b * P:(b + 1) * P],
                    rhs=xT_sb[:, et, :],
                    start=(et == 0),
                    stop=(et == ET - 1),
                )
            m, r = divmod(b, 3)
            if r == 0:
                # q_2m -> both halves of qT; k_2m -> hi half of kT slot m
                copy(ps_b, (0, HD), b, qT[0:HD, :, 2 * m, :], eng)
                copy(ps_b, (0, HD), b, qT[HD:P, :, 2 * m, :], eng)
                copy(ps_b, (HD, P), b, kT[HD:P, :, m, :], eng)
            elif r == 1:
                vt_new = vtmp.tile([HD, NH * P], BF16, tag="vt")
                vts[2 * m] = vt_new
                copy(ps_b, (0, HD), b, vt_new[:, :], eng)
                copy(ps_b, (HD, P), b, qT[0:HD, :, 2 * m + 1, :], eng)
                copy(ps_b, (HD, P), b, qT[HD:P, :, 2 * m + 1, :], eng)
                finish_v(2 * m, eng)
            else:
                copy(ps_b, (0, HD), b, kT[0:HD, :, m, :], eng)
                vt_new = vtmp.tile([HD, NH * P], BF16, tag="vt")
                vts[2 * m + 1] = vt_new
                copy(ps_b, (HD, P), b, vt_new[:, :], eng)
                finish_v(2 * m + 1, eng)

        # ---- attention helpers ---------------------------------------------
        def score_pair(i, c, t):
            """Row-tiled concurrent scores for kt=2t (hi half) and kt=2t+1
            (lo half).  Returns (slot_even, slot_odd)."""
            slotB = ps.tile([P, 2, 512], F32, tag="slot")  # kt = 2t
            slotA = ps.tile([P, 2, 512], F32, tag="slot")  # kt = 2t+1
            for cc in range(2):
                q_ap_hi = qT[HD:P, i, c * 8 + cc * 4:c * 8 + (cc + 1) * 4, :]
                q_ap_lo = qT[0:HD, i, c * 8 + cc * 4:c * 8 + (cc + 1) * 4, :]
                nc.tensor.matmul(
                    slotB[:, cc, :],
                    lhsT=kT[HD:P, i, t, :],
                    rhs=q_ap_hi,
                    start=True, stop=True,
                    tile_position=(HD, 0),
                )
                nc.tensor.matmul(
                    slotA[:, cc, :],
                    lhsT=kT[0:HD, i, t, :],
                    rhs=q_ap_lo,
                    start=True, stop=True,
                    tile_position=(0, 0),
                )
            return slotB, slotA

        def exp_kt(slot):
            pT = ppool.tile([P, CH], BF16, tag="pT")
            nc.scalar.activation(
                out=pT, in_=slot[:, :, :].rearrange("p a b -> p (a b)"),
                func=EXP, scale=0.125,
            )
            return pT

        def av_kt(i, kt, pT, pctx):
            for cc in range(2):
                nc.tensor.matmul(
                    pctx[:, cc * 512:(cc + 1) * 512],
                    lhsT=vaug[:, i, kt, :],
                    rhs=pT[:, cc * 512:(cc + 1) * 512],
                    start=(kt == 0),
                    stop=(kt == NJ - 1),
                )

        def chunk_end(i, c, pctx):
            rrow = rpool.tile([1, CH], F32, tag="rrow")
            nc.vector.reciprocal(out=rrow, in_=pctx[HD:HD + 1, :])
            rb = rpool.tile([HD, CH], F32, tag="rb")
            nc.gpsimd.partition_broadcast(rb, rrow, channels=HD)
            phalf = (i % 2) * HD
            nc.vector.tensor_tensor(
                out=ctxT[phalf:phalf + HD, i // 2, c * CH:(c + 1) * CH],
                in0=pctx[0:HD, :],
                in1=rb,
                op=mybir.AluOpType.mult,
            )

        # ---- out-projection -------------------------------------------------
        # out_part[s', f] = sum_{d'} ctxT[d', s''] * woutT[d', f], written to
        # DRAM (bf16) with the s'' -> s' = 16r + j permutation in the AP.
        out_view = outp.rearrange("(r six) f -> six r f", six=NJ)  # [16,128,1024]

        def outproj_st(st, tail):
            o_sb = opool.tile([P, E], BF16, tag="osb")
            for fc in range(2):
                ps_o = ps.tile([P, 512], F32, tag="slot")
                for ktile in range(2):
                    nc.tensor.matmul(
                        ps_o,
                        lhsT=ctxT[:, ktile, st * P:(st + 1) * P],
                        rhs=woutT_sb[:, ktile, fc * 512:(fc + 1) * 512],
                        start=(ktile == 0),
                        stop=(ktile == 1),
                    )
                if fc == 0 and tail:
                    nc.scalar.copy(out=o_sb[:, 0:512], in_=ps_o)
                else:
                    nc.vector.tensor_copy(
                        out=o_sb[:, fc * 512:(fc + 1) * 512], in_=ps_o
                    )
            eng = nc.sync if st % 2 == 0 else nc.gpsimd
            eng.dma_start(out=out_view[st, :, :], in_=o_sb)

        # ---- emission ------------------------------------------------------
        for b in range(12):
            proj_block(b, "act")

        # outproj of chunk 0 (st 0..7) is interleaved into chunk 1 attention
        # of heads 0 and 1, one st-tile per kt-pair.
        for c in range(2):
            for i in range(NH):
                pctx = ps.tile([HD + 1, CH], F32, tag="ctx")
                for t in range(8):
                    slotB, slotA = score_pair(i, c, t)
                    if c == 0 and i == 0 and t < 6:
                        proj_block(12 + 2 * t, "dve")
                        proj_block(13 + 2 * t, "dve")
                    pT_e = exp_kt(slotB)
                    pT_o = exp_kt(slotA)
                    av_kt(i, 2 * t, pT_e, pctx)
                    av_kt(i, 2 * t + 1, pT_o, pctx)
                    if c == 1 and i == 0:
                        outproj_st(t, tail=False)
                chunk_end(i, c, pctx)

        for st in range(8, NJ):
            outproj_st(st, tail=True)


def build_nc():
    nc = bacc.Bacc("TRN2", target_bir_lowering=False, debug=False, num_devices=8)
    xT = nc.declare_dram_parameter("xT", [ET, P, NH * P], BF16, isOutput=False)
    wqkvT = nc.declare_dram_parameter("wqkvT", [ET, P, 3 * E], BF16, isOutput=False)
    woutT = nc.declare_dram_parameter("woutT", [2, P, E], BF16, isOutput=False)
    bblk = nc.declare_dram_parameter("bblk", [P, 24], F32, isOutput=False)
    outp = nc.declare_dram_parameter("out_part", [S, E], BF16, isOutput=True)
    with tile.TileContext(nc) as tc:
        _emit(nc, tc, xT, wqkvT, woutT, bblk, outp)
    nc.compile()
    return nc


def make_in_maps(x, W_qkv, b_qkv, W_out):
    import ml_dtypes
    bf16 = ml_dtypes.bfloat16
    x = np.asarray(x, np.float32)
    # [ET, P, 3E]: wqkvT[et, p, f] = W_qkv.T[et*128+p, f], cast to bf16
    wqkvT = np.ascontiguousarray(
        np.asarray(W_qkv, np.float32).T.reshape(ET, P, 3 * E)
    ).astype(bf16)
    woutT = np.ascontiguousarray(np.asarray(W_out, np.float32).T)
    b_qkv = np.asarray(b_qkv, np.float32)
    bblk = np.ascontiguousarray(np.asarray(b_qkv, np.float32).reshape(24, P).T)
    in_maps = []
    for core in range(8):
        b, g = divmod(core, 4)
        in_maps.append({
            "xT": np.ascontiguousarray(
                x[b, 512 * g:512 * (g + 1), :].T.reshape(ET, P, NH * P)
            ).astype(bf16),
            "wqkvT": wqkvT,
            "woutT": np.ascontiguousarray(
                woutT[256 * g:256 * (g + 1), :].reshape(2, P, E)
            ).astype(bf16),
            "bblk": bblk,
        })
    return in_maps


def kernel(x, W_qkv, b_qkv, W_out, b_out):
    global _NC_CACHE, _LAST_RESULT
    if _NC_CACHE is None:
        _NC_CACHE = build_nc()
    in_maps = make_in_maps(x, W_qkv, b_qkv, W_out)
    _LAST_RESULT = run_bass_kernel_spmd(_NC_CACHE, in_maps, list(range(8)))
    res = _LAST_RESULT.results
    b_out = np.asarray(b_out, np.float32)
    out = np.empty((B, S, E), np.float32)
    for b in range(B):
        acc = np.asarray(res[4 * b]["out_part"], np.float32).copy()
        for g in range(1, 4):
            acc += np.asarray(res[4 * b + g]["out_part"], np.float32)
        out[b] = acc + b_out
    return out


# revision 23
# speedup vs baseline: 1.1407x; 1.1407x over previous
"""Trainium2 Bass kernel for the torch-faithful MultiHeadAttention module.

Math (validated vs the jax reference):
  qkv = x @ W_qkv.T + b_qkv                    # [B, S, 3E]
  qkv.view(B, H, -1, 3*hd)  is a PLAIN reshape, so "head" h is really the
  sequence block s in [128h, 128h+128), and within a head the 2048 rows are
  s' = (s%128)*16 + j with j = f//192; q/k/v are column slices of each
  192-wide block j.
  score = q @ k.T / 8 ; softmax ; context ; out = context' @ W_out.T + b_out

Sharding (8 cores): data-parallel over batch (4 cores per batch element),
head-parallel within the group (4 heads per core).  Each core computes its
heads' attention entirely on-chip (flash style, no HBM score matrix) and a
partial out-projection over its 256 context columns; the host sums the 4
partials per batch element (a pure unshard/reduce step) and adds b_out.

Internally each head uses the s'' = j*128 + r ordering (a permutation of
s'); the permutation is undone for free in the final strided DMA to DRAM.

Performance structure (vs the first-generation kernel):
- Scores run as ROW-TILED pairs: the K=64 contraction only fills half the
  PE array, so two kt-tiles (one odd-j with K at partitions 0-63, one
  even-j at 64-127) execute concurrently via tile_position=(0,0)/(64,0).
  The block structure of W_qkv^T makes the k parity split land on the
  right partition halves for free; q is written to both halves.
- Softmax normalization: the ones-column of v_aug gives the row sums l as
  PSUM row 64; 1/l comes from a single DVE reciprocal on that row, is
  broadcast across partitions by an SBUF->SBUF DMA, and one fused DVE
  multiply produces normalized ctx^T directly from PSUM.  No DRAM bounce,
  no PE transposes at chunk boundaries.
- Chunk-major (c, head) order; the out-projection of the first q-chunk is
  interleaved into the second chunk's attention; output partials are bf16.
"""

import numpy as np

import concourse.bass as bass
import concourse.mybir as mybir
import concourse.tile as tile
from concourse import bacc
from concourse.bass_utils import run_bass_kernel_spmd
from concourse.masks import make_identity

B, S, E = 2, 2048, 1024
H, HD = 16, 64
NH = 4   # heads per core
NJ = 16  # 192-wide column blocks in 3E == kt tiles per head
P = 128
ET = E // P  # 8 contraction tiles of 128
CH = 1024    # q-chunk width
F32 = mybir.dt.float32
BF16 = mybir.dt.bfloat16
EXP = mybir.ActivationFunctionType.Exp
IDENT = mybir.ActivationFunctionType.Identity

_NC_CACHE = None
_LAST_RESULT = None  # BassKernelResults of the most recent run (for test harness)


def _emit(nc, tc, xT, wqkvT, woutT, bblk, outp):
    import contextlib

    with contextlib.ExitStack() as ctx:
        ctx.enter_context(
            nc.allow_low_precision(reason="bf16 matmul operands")
        )
        const = ctx.enter_context(tc.tile_pool(name="const", bufs=1))
        vtmp = ctx.enter_context(tc.tile_pool(name="vtmp", bufs=2))
        ppool = ctx.enter_context(tc.tile_pool(name="probs", bufs=4))
        rpool = ctx.enter_context(tc.tile_pool(name="recip", bufs=2))
        opool = ctx.enter_context(tc.tile_pool(name="ostage", bufs=2))
        # PSUM: "slot" ring 3 x [128,2,512]f32 (6 banks) shared by scores /
        # proj / v-transposes / outproj, + "ctx" 1 x [65,1024]f32 (2 banks).
        # Total exactly 8 banks.
        ps = ctx.enter_context(tc.tile_pool(name="ps", bufs=3, space="PSUM"))

        # ---- resident tiles -------------------------------------------------
        xT_sb = const.tile([P, ET, NH * P], BF16, tag="xT")  # [128, 8, 512]
        for et in range(ET):
            nc.sync.dma_start(out=xT_sb[:, et, :], in_=xT[et, :, :])

        bblk_sb = const.tile([P, 24], F32, tag="bblk")
        nc.sync.dma_start(out=bblk_sb, in_=bblk[:, :])

        # W_qkv^T fully resident in bf16: [128, 8, 3072] = 48KB/partition.
        # One 3-D DMA per 128-wide column block, in consumption order.
        wq_all = const.tile([P, ET, 3 * E], BF16, tag="wq")
        wq_src = wqkvT.rearrange("et p f -> p et f")
        for b in range(24):
            nc.sync.dma_start(
                out=wq_all[:, :, b * P:(b + 1) * P],
                in_=wq_src[:, :, b * P:(b + 1) * P],
            )

        woutT_sb = const.tile([P, 2, E], BF16, tag="woutT")  # [128, 2, 1024]
        for t in range(2):
            nc.gpsimd.dma_start(out=woutT_sb[:, t, :], in_=woutT[t, :, :])

        ident = const.tile([P, P], BF16, tag="ident")
        make_identity(nc, ident)

        # q^T, replicated on both partition halves: [128, nh, nj, 128]
        qT = const.tile([P, NH, NJ, P], BF16, tag="qT")
        # k^T parity-split: partitions 0-63 hold k_{2t+1}, 64-127 hold k_{2t}
        kT = const.tile([P, NH, NJ // 2, P], BF16, tag="kT")
        # v_aug per head per j-block: [128 rows, 128 cols]: column 0 is the
        # ones column (so the softmax row-sum l lands on PSUM partition 0,
        # where gpsimd.partition_broadcast reads), columns 1-63 are zero,
        # and columns 64-127 hold v (so ctx lands on partitions 64-127, an
        # aligned engine base).
        vaug = const.tile([P, NH, NJ, P], BF16, tag="vaug")
        nc.vector.memset(vaug[:, :, :, 0:HD], 0.0)
        nc.vector.memset(vaug[:, :, :, 0:1], 1.0)
        # normalized context^T: K-tile t holds heads (2t, 2t+1) on halves
        ctxT = const.tile([P, 2, S], BF16, tag="ctxT")

        # ---- qkv projection helpers ----------------------------------------
        # Block pattern per two j:
        #   b=3m: [q_2m | k_2m]  b=3m+1: [v_2m | q_2m+1]  b=3m+2: [k_2m+1 | v_2m+1]
        # ps rows 0-63 are the first half (q/v/k resp.), 64-127 the second.
        vts = {}

        def copy(ps_b, rows, bcol, out_ap, eng):
            src = ps_b[rows[0]:rows[1], :]
            if out_ap.shape[-1] == P and len(out_ap.shape) > 2:
                src = src.rearrange("d (nh p) -> d nh p", p=P)
            bias = bblk_sb[rows[0]:rows[1], bcol:bcol + 1]
            if eng == "act":
                nc.scalar.activation(out=out_ap, in_=src, func=IDENT, bias=bias)
            else:
                nc.vector.tensor_scalar_add(out=out_ap, in0=src, scalar1=bias)

        def finish_v(j, eng):
            vt_j = vts.pop(j)
            ps_tr = ps.tile([P, NH, HD], BF16, tag="slot")
            for i in range(NH):
                nc.tensor.transpose(
                    ps_tr[:, i, :], vt_j[:, i * P:(i + 1) * P],
                    ident[0:HD, 0:HD],
                )
            if eng == "act":
                nc.scalar.copy(out=vaug[:, :, j, HD:P], in_=ps_tr)
            else:
                nc.vector.tensor_copy(out=vaug[:, :, j, HD:P], in_=ps_tr)

        def proj_block(b, eng):
            ps_b = ps.tile([P, NH * P], F32, tag="slot")
            for et in range(ET):
                nc.tensor.matmul(
                    ps_b,
                    lhsT=wq_all[:, et, b * P:(b + 1) * P],
                    rhs=xT_sb[:, et, :],
                    start=(et == 0),
                    stop=(et == ET - 1),
                )
            m, r = divmod(b, 3)
            if r == 0:
                # q_2m -> both halves of qT; k_2m -> hi half of kT slot m
                copy(ps_b, (0, HD), b, qT[0:HD, :, 2 * m, :], eng)
                copy(ps_b, (0, HD), b, qT[HD:P, :, 2 * m, :], eng)
                copy(ps_b, (HD, P), b, kT[HD:P, :, m, :], eng)
            elif r == 1:
                vt_new = vtmp.tile([HD, NH * P], BF16, tag="vt")
                vts[2 * m] = vt_new
                copy(ps_b, (0, HD), b, vt_new[:, :], eng)
                copy(ps_b, (HD, P), b, qT[0:HD, :, 2 * m + 1, :], eng)
                copy(ps_b, (HD, P), b, qT[HD:P, :, 2 * m + 1, :], eng)
                finish_v(2 * m, eng)
            else:
                copy(ps_b, (0, HD), b, kT[0:HD, :, m, :], eng)
                vt_new = vtmp.tile([HD, NH * P], BF16, tag="vt")
                vts[2 * m + 1] = vt_new
                copy(ps_b, (HD, P), b, vt_new[:, :], eng)
                finish_v(2 * m + 1, eng)

        # ---- attention helpers ---------------------------------------------
        def score_half(i, c, t, hi):
            """Scores for kt=2t (hi partition half) or kt=2t+1 (lo half).
            The two halves use disjoint PE row-groups (tile_position), so
            adjacent hi/lo matmuls can run concurrently."""
            slot = ps.tile([P, 2, 512], F32, tag="slot")
            lo, po = (HD, P) if hi else (0, HD)
            for cc in range(2):
                nc.tensor.matmul(
                    slot[:, cc, :],
                    lhsT=kT[lo:po, i, t, :],
                    rhs=qT[lo:po, i, c * 8 + cc * 4:c * 8 + (cc + 1) * 4, :],
                    start=True, stop=True,
                    tile_position=(lo, 0),
                )
            return slot

        def exp_kt(slot):
            pT = ppool.tile([P, CH], BF16, tag="pT", bufs=6)
            nc.scalar.activation(
                out=pT, in_=slot[:, :, :].rearrange("p a b -> p (a b)"),
                func=EXP, scale=0.125,
            )
            return pT

        def av_kt(i, kt, pT, pctx):
            for cc in range(2):
                nc.tensor.matmul(
                    pctx[:, cc * 512:(cc + 1) * 512],
                    lhsT=vaug[:, i, kt, :],
                    rhs=pT[:, cc * 512:(cc + 1) * 512],
                    start=(kt == 0),
                    stop=(kt == NJ - 1),
                )

        def chunk_end(i, c, pctx):
            # [l; 0...; ctx^T] -> SBUF (frees pctx), broadcast l (partition
            # 0) across partitions, reciprocal, then one multiply produces
            # normalized ctx^T in ctxT.
            csb = rpool.tile([P, CH], F32, tag="csb")
            nc.vector.tensor_copy(out=csb, in_=pctx)
            lb = rpool.tile([HD, CH], F32, tag="lb")
            nc.gpsimd.partition_broadcast(lb, csb[0:1, :], channels=HD)
            rinv = rpool.tile([P, CH], F32, tag="rinv")
            nc.vector.reciprocal(out=rinv[HD:P, :], in_=lb)
            phalf = (i % 2) * HD
            nc.vector.tensor_tensor(
                out=ctxT[phalf:phalf + HD, i // 2, c * CH:(c + 1) * CH],
                in0=csb[HD:P, :],
                in1=rinv[HD:P, :],
                op=mybir.AluOpType.mult,
            )

        # ---- out-projection -------------------------------------------------
        # out_part[s', f] = sum_{d'} ctxT[d', s''] * woutT[d', f], written to
        # DRAM (bf16) with the s'' -> s' = 16r + j permutation in the AP.
        out_view = outp.rearrange("(r six) f -> six r f", six=NJ)  # [16,128,1024]

        def outproj_st(st, tail):
            o_sb = opool.tile([P, E], BF16, tag="osb")
            for fc in range(2):
                ps_o = ps.tile([P, 512], F32, tag="slot")
                for ktile in range(2):
                    nc.tensor.matmul(
                        ps_o,
                        lhsT=ctxT[:, ktile, st * P:(st + 1) * P],
                        rhs=woutT_sb[:, ktile, fc * 512:(fc + 1) * 512],
                        start=(ktile == 0),
                        stop=(ktile == 1),
                    )
                if fc == 0 and tail:
                    nc.scalar.copy(out=o_sb[:, 0:512], in_=ps_o)
                else:
                    nc.vector.tensor_copy(
                        out=o_sb[:, fc * 512:(fc + 1) * 512], in_=ps_o
                    )
            eng = nc.sync if st % 2 == 0 else nc.gpsimd
            eng.dma_start(out=out_view[st, :, :], in_=o_sb)

        # ---- emission ------------------------------------------------------
        for b in range(12):
            proj_block(b, "act")

        # Attention, software-pipelined with a global A@V queue: the A@V for
        # kt is emitted ~1.5 pairs after its exp, so the PE never waits on
        # the freshest exp and the 3-deep score-slot ring never stalls.
        # outproj of chunk 0 (st 0..7) is interleaved into chunk 1 attention
        # of heads 0 and 1.
        avq = []  # FIFO of (i, c, kt, pT)
        state = {"pctx": None}

        def pop_av():
            i, c, kt, pT = avq.pop(0)
            if kt == 0:
                state["pctx"] = ps.tile([P, CH], F32, tag="ctx", bufs=1, name="pctx")
            av_kt(i, kt, pT, state["pctx"])
            if kt == NJ - 1:
                chunk_end(i, c, state["pctx"])

        for c in range(2):
            for i in range(NH):
                for t in range(8):
                    slotB = score_half(i, c, t, hi=True)
                    if c == 0 and i == 0 and t < 6:
                        proj_block(12 + 2 * t, "dve")
                        proj_block(13 + 2 * t, "dve")
                    while len(avq) > 3:
                        pop_av()
                    slotA = score_half(i, c, t, hi=False)
                    pT_e = exp_kt(slotB)
                    pT_o = exp_kt(slotA)
                    avq.append((i, c, 2 * t, pT_e))
                    avq.append((i, c, 2 * t + 1, pT_o))
                    if c == 1 and i < 2 and t % 2 == 1:
                        # chunk-0 ctxT of ALL heads must be complete first
                        while any(e[1] == 0 for e in avq):
                            pop_av()
                        outproj_st(i * 4 + t // 2, tail=False)
        while avq:
            pop_av()

        for st in range(8, NJ):
            outproj_st(st, tail=True)


def build_nc():
    nc = bacc.Bacc("TRN2", target_bir_lowering=False, debug=False, num_devices=8)
    xT = nc.declare_dram_parameter("xT", [ET, P, NH * P], BF16, isOutput=False)
    wqkvT = nc.declare_dram_parameter("wqkvT", [ET, P, 3 * E], BF16, isOutput=False)
    woutT = nc.declare_dram_parameter("woutT", [2, P, E], BF16, isOutput=False)
    bblk = nc.declare_dram_parameter("bblk", [P, 24], F32, isOutput=False)
    outp = nc.declare_dram_parameter("out_part", [S, E], BF16, isOutput=True)
    with tile.TileContext(nc) as tc:
        _emit(nc, tc, xT, wqkvT, woutT, bblk, outp)
    nc.compile()
    return nc


def make_in_maps(x, W_qkv, b_qkv, W_out):
    import ml_dtypes
    bf16 = ml_dtypes.bfloat16
    x = np.asarray(x, np.float32)
    # [ET, P, 3E]: wqkvT[et, p, f] = W_qkv.T[et*128+p, f], cast to bf16
    wqkvT = np.ascontiguousarray(
        np.asarray(W_qkv, np.float32).T.reshape(ET, P, 3 * E)
    ).astype(bf16)
    woutT = np.ascontiguousarray(np.asarray(W_out, np.float32).T)
    b_qkv = np.asarray(b_qkv, np.float32)
    bblk = np.ascontiguousarray(np.asarray(b_qkv, np.float32).reshape(24, P).T)
    in_maps = []
    for core in range(8):
        b, g = divmod(core, 4)
        in_maps.append({
            "xT": np.ascontiguousarray(
                x[b, 512 * g:512 * (g + 1), :].T.reshape(ET, P, NH * P)
            ).astype(bf16),
            "wqkvT": wqkvT,
            "woutT": np.ascontiguousarray(
                woutT[256 * g:256 * (g + 1), :].reshape(2, P, E)
            ).astype(bf16),
            "bblk": bblk,
        })
    return in_maps


def kernel(x, W_qkv, b_qkv, W_out, b_out):
    global _NC_CACHE, _LAST_RESULT
    if _NC_CACHE is None:
        _NC_CACHE = build_nc()
    in_maps = make_in_maps(x, W_qkv, b_qkv, W_out)
    _LAST_RESULT = run_bass_kernel_spmd(_NC_CACHE, in_maps, list(range(8)))
    res = _LAST_RESULT.results
    b_out = np.asarray(b_out, np.float32)
    out = np.empty((B, S, E), np.float32)
    for b in range(B):
        acc = np.asarray(res[4 * b]["out_part"], np.float32).copy()
        for g in range(1, 4):
            acc += np.asarray(res[4 * b + g]["out_part"], np.float32)
        out[b] = acc + b_out
    return out


# revision 25
# speedup vs baseline: 1.6146x; 1.4154x over previous
"""Trainium2 Bass kernel for the torch-faithful MultiHeadAttention module.

Math (validated vs the jax reference):
  qkv = x @ W_qkv.T + b_qkv                    # [B, S, 3E]
  qkv.view(B, H, -1, 3*hd)  is a PLAIN reshape, so "head" h is really the
  sequence block s in [128h, 128h+128), and within a head the 2048 rows are
  s' = (s%128)*16 + j with j = f//192; q/k/v are column slices of each
  192-wide block j.
  score = q @ k.T / 8 ; softmax ; context ; out = context' @ W_out.T + b_out

Sharding (8 cores): data-parallel over batch (4 cores per batch element),
head-parallel within the group (4 heads per core).  Each core computes its
heads' attention entirely on-chip (flash style, no HBM score matrix) and a
partial out-projection over its 256 context columns; the host sums the 4
partials per batch element (a pure unshard/reduce step) and adds b_out.

Internally each head uses the s'' = j*128 + r ordering (a permutation of
s'); the permutation is undone for free in the final strided DMA to DRAM.

Performance structure (vs the first-generation kernel):
- Scores run as ROW-TILED pairs: the K=64 contraction only fills half the
  PE array, so two kt-tiles (one odd-j with K at partitions 0-63, one
  even-j at 64-127) execute concurrently via tile_position=(0,0)/(64,0).
  The block structure of W_qkv^T makes the k parity split land on the
  right partition halves for free; q is written to both halves.
- Softmax normalization: the ones-column of v_aug gives the row sums l as
  PSUM row 64; 1/l comes from a single DVE reciprocal on that row, is
  broadcast across partitions by an SBUF->SBUF DMA, and one fused DVE
  multiply produces normalized ctx^T directly from PSUM.  No DRAM bounce,
  no PE transposes at chunk boundaries.
- Chunk-major (c, head) order; the out-projection of the first q-chunk is
  interleaved into the second chunk's attention; output partials are bf16.
"""

import numpy as np

import concourse.bass as bass
import concourse.mybir as mybir
import concourse.tile as tile
from concourse import bacc
from concourse.bass_utils import run_bass_kernel_spmd
from concourse.masks import make_identity

B, S, E = 2, 2048, 1024
H, HD = 16, 64
NH = 4   # heads per core
NJ = 16  # 192-wide column blocks in 3E == kt tiles per head
P = 128
ET = E // P  # 8 contraction tiles of 128
CH = 1024    # q-chunk width
F32 = mybir.dt.float32
BF16 = mybir.dt.bfloat16
EXP = mybir.ActivationFunctionType.Exp
IDENT = mybir.ActivationFunctionType.Identity

_NC_CACHE = None
_LAST_RESULT = None  # BassKernelResults of the most recent run (for test harness)


def _emit(nc, tc, xT, wqkvT, woutT, bblk, outp):
    import contextlib

    with contextlib.ExitStack() as ctx:
        ctx.enter_context(
            nc.allow_low_precision(reason="bf16 matmul operands")
        )
        const = ctx.enter_context(tc.tile_pool(name="const", bufs=1))
        vtmp = ctx.enter_context(tc.tile_pool(name="vtmp", bufs=2))
        ppool = ctx.enter_context(tc.tile_pool(name="probs", bufs=4))
        rpool = ctx.enter_context(tc.tile_pool(name="recip", bufs=2))
        opool = ctx.enter_context(tc.tile_pool(name="ostage", bufs=2))
        # PSUM: "slot" ring 3 x [128,2,512]f32 (6 banks) shared by scores /
        # proj / v-transposes / outproj, + "ctx" 1 x [65,1024]f32 (2 banks).
        # Total exactly 8 banks.
        ps = ctx.enter_context(tc.tile_pool(name="ps", bufs=3, space="PSUM"))

        # ---- resident tiles -------------------------------------------------
        xT_sb = const.tile([P, ET, NH * P], BF16, tag="xT")  # [128, 8, 512]
        for et in range(ET):
            nc.sync.dma_start(out=xT_sb[:, et, :], in_=xT[et, :, :])

        bblk_sb = const.tile([P, 24], F32, tag="bblk")
        nc.sync.dma_start(out=bblk_sb, in_=bblk[:, :])

        # W_qkv^T fully resident in bf16: [128, 8, 3072] = 48KB/partition.
        # One 3-D DMA per 128-wide column block, in consumption order.
        wq_all = const.tile([P, ET, 3 * E], BF16, tag="wq")
        wq_src = wqkvT.rearrange("et p f -> p et f")
        for b in range(24):
            nc.sync.dma_start(
                out=wq_all[:, :, b * P:(b + 1) * P],
                in_=wq_src[:, :, b * P:(b + 1) * P],
            )

        woutT_sb = const.tile([P, 2, E], BF16, tag="woutT")  # [128, 2, 1024]
        for t in range(2):
            nc.gpsimd.dma_start(out=woutT_sb[:, t, :], in_=woutT[t, :, :])

        ident = const.tile([P, P], BF16, tag="ident")
        make_identity(nc, ident)

        # q^T, replicated on both partition halves: [128, nh, nj, 128]
        qT = const.tile([P, NH, NJ, P], BF16, tag="qT")
        # k^T parity-split: partitions 0-63 hold k_{2t+1}, 64-127 hold k_{2t}
        kT = const.tile([P, NH, NJ // 2, P], BF16, tag="kT")
        # v_aug per head per j-block: [128 rows, 128 cols]: column 0 is the
        # ones column (so the softmax row-sum l lands on PSUM partition 0,
        # where gpsimd.partition_broadcast reads), columns 1-63 are zero,
        # and columns 64-127 hold v (so ctx lands on partitions 64-127, an
        # aligned engine base).
        vaug = const.tile([P, NH, NJ, P], BF16, tag="vaug")
        nc.vector.memset(vaug[:, :, :, 0:HD], 0.0)
        nc.vector.memset(vaug[:, :, :, 0:1], 1.0)
        # normalized context^T: K-tile t holds heads (2t, 2t+1) on halves
        ctxT = const.tile([P, 2, S], BF16, tag="ctxT")

        # ---- qkv projection helpers ----------------------------------------
        # Block pattern per two j:
        #   b=3m: [q_2m | k_2m]  b=3m+1: [v_2m | q_2m+1]  b=3m+2: [k_2m+1 | v_2m+1]
        # ps rows 0-63 are the first half (q/v/k resp.), 64-127 the second.
        vts = {}

        def copy(ps_b, rows, bcol, out_ap, eng):
            src = ps_b[rows[0]:rows[1], :]
            if out_ap.shape[-1] == P and len(out_ap.shape) > 2:
                src = src.rearrange("d (nh p) -> d nh p", p=P)
            bias = bblk_sb[rows[0]:rows[1], bcol:bcol + 1]
            if eng == "act":
                nc.scalar.activation(out=out_ap, in_=src, func=IDENT, bias=bias)
            else:
                nc.vector.tensor_scalar_add(out=out_ap, in0=src, scalar1=bias)

        def finish_v(j, eng):
            vt_j = vts.pop(j)
            ps_tr = ps.tile([P, NH, HD], BF16, tag="slot")
            for i in range(NH):
                nc.tensor.transpose(
                    ps_tr[:, i, :], vt_j[:, i * P:(i + 1) * P],
                    ident[0:HD, 0:HD],
                )
            if eng == "act":
                nc.scalar.copy(out=vaug[:, :, j, HD:P], in_=ps_tr)
            else:
                nc.vector.tensor_copy(out=vaug[:, :, j, HD:P], in_=ps_tr)

        def proj_block(b, eng):
            ps_b = ps.tile([P, NH * P], F32, tag="slot")
            for et in range(ET):
                nc.tensor.matmul(
                    ps_b,
                    lhsT=wq_all[:, et, b * P:(b + 1) * P],
                    rhs=xT_sb[:, et, :],
                    start=(et == 0),
                    stop=(et == ET - 1),
                )
            m, r = divmod(b, 3)
            if r == 0:
                # q_2m -> both halves of qT; k_2m -> hi half of kT slot m
                copy(ps_b, (0, HD), b, qT[0:HD, :, 2 * m, :], eng)
                copy(ps_b, (0, HD), b, qT[HD:P, :, 2 * m, :], eng)
                copy(ps_b, (HD, P), b, kT[HD:P, :, m, :], eng)
            elif r == 1:
                vt_new = vtmp.tile([HD, NH * P], BF16, tag="vt")
                vts[2 * m] = vt_new
                copy(ps_b, (0, HD), b, vt_new[:, :], eng)
                copy(ps_b, (HD, P), b, qT[0:HD, :, 2 * m + 1, :], eng)
                copy(ps_b, (HD, P), b, qT[HD:P, :, 2 * m + 1, :], eng)
                finish_v(2 * m, eng)
            else:
                copy(ps_b, (0, HD), b, kT[0:HD, :, m, :], eng)
                vt_new = vtmp.tile([HD, NH * P], BF16, tag="vt")
                vts[2 * m + 1] = vt_new
                copy(ps_b, (HD, P), b, vt_new[:, :], eng)
                finish_v(2 * m + 1, eng)

        # ---- attention helpers ---------------------------------------------
        def score_half(i, c, t, hi):
            """Scores for kt=2t (hi partition half) or kt=2t+1 (lo half).
            The two halves use disjoint PE row-groups (tile_position), so
            adjacent hi/lo matmuls can run concurrently."""
            slot = ps.tile([P, 2, 512], F32, tag="slot")
            lo, po = (HD, P) if hi else (0, HD)
            for cc in range(2):
                nc.tensor.matmul(
                    slot[:, cc, :],
                    lhsT=kT[lo:po, i, t, :],
                    rhs=qT[lo:po, i, c * 8 + cc * 4:c * 8 + (cc + 1) * 4, :],
                    start=True, stop=True,
                    tile_position=(lo, 0),
                )
            return slot

        def exp_kt(slot):
            pT = ppool.tile([P, CH], BF16, tag="pT", bufs=6)
            nc.scalar.activation(
                out=pT, in_=slot[:, :, :].rearrange("p a b -> p (a b)"),
                func=EXP, scale=0.125,
            )
            return pT

        def av_kt(i, kt, pT, pctx):
            for cc in range(2):
                nc.tensor.matmul(
                    pctx[:, cc * 512:(cc + 1) * 512],
                    lhsT=vaug[:, i, kt, :],
                    rhs=pT[:, cc * 512:(cc + 1) * 512],
                    start=(kt == 0),
                    stop=(kt == NJ - 1),
                )

        def chunk_end(i, c, pctx):
            # [l; 0...; ctx^T] -> SBUF (frees pctx), broadcast l (partition
            # 0) across partitions, reciprocal, then one multiply produces
            # normalized ctx^T in ctxT.
            csb = rpool.tile([P, CH], F32, tag="csb")
            nc.vector.tensor_copy(out=csb, in_=pctx)
            rrow = rpool.tile([1, CH], F32, tag="rrow")
            nc.vector.reciprocal_approx_fast(out=rrow, in_=csb[0:1, :])
            rb = rpool.tile([P, CH], F32, tag="rb")
            nc.gpsimd.partition_broadcast(rb, rrow, channels=P)
            phalf = (i % 2) * HD
            nc.vector.tensor_tensor(
                out=ctxT[phalf:phalf + HD, i // 2, c * CH:(c + 1) * CH],
                in0=csb[HD:P, :],
                in1=rb[HD:P, :],
                op=mybir.AluOpType.mult,
            )

        # ---- out-projection -------------------------------------------------
        # out_part[s', f] = sum_{d'} ctxT[d', s''] * woutT[d', f], written to
        # DRAM (bf16) with the s'' -> s' = 16r + j permutation in the AP.
        out_view = outp.rearrange("(r six) f -> six r f", six=NJ)  # [16,128,1024]

        def outproj_st(st, tail):
            o_sb = opool.tile([P, E], BF16, tag="osb")
            for fc in range(2):
                ps_o = ps.tile([P, 512], F32, tag="slot")
                for ktile in range(2):
                    nc.tensor.matmul(
                        ps_o,
                        lhsT=ctxT[:, ktile, st * P:(st + 1) * P],
                        rhs=woutT_sb[:, ktile, fc * 512:(fc + 1) * 512],
                        start=(ktile == 0),
                        stop=(ktile == 1),
                    )
                if fc == 0 and tail:
                    nc.scalar.copy(out=o_sb[:, 0:512], in_=ps_o)
                else:
                    nc.vector.tensor_copy(
                        out=o_sb[:, fc * 512:(fc + 1) * 512], in_=ps_o
                    )
            eng = nc.sync if st % 2 == 0 else nc.gpsimd
            eng.dma_start(out=out_view[st, :, :], in_=o_sb)

        # ---- emission ------------------------------------------------------
        for b in range(12):
            proj_block(b, "act")

        # Attention, software-pipelined with a global A@V queue: the A@V for
        # kt is emitted ~1.5 pairs after its exp, so the PE never waits on
        # the freshest exp and the 3-deep score-slot ring never stalls.
        # outproj of chunk 0 (st 0..7) is interleaved into chunk 1 attention
        # of heads 0 and 1.
        avq = []  # FIFO of (i, c, kt, pT)
        state = {"pctx": None}

        def pop_av():
            i, c, kt, pT = avq.pop(0)
            if kt == 0:
                state["pctx"] = ps.tile([P, CH], F32, tag="ctx", bufs=1, name="pctx")
            av_kt(i, kt, pT, state["pctx"])
            if kt == NJ - 1:
                chunk_end(i, c, state["pctx"])

        for c in range(2):
            for i in range(NH):
                for t in range(8):
                    slotB = score_half(i, c, t, hi=True)
                    if c == 0 and i == 0 and t < 6:
                        proj_block(12 + 2 * t, "dve")
                        proj_block(13 + 2 * t, "dve")
                    while len(avq) > 3:
                        pop_av()
                    slotA = score_half(i, c, t, hi=False)
                    pT_e = exp_kt(slotB)
                    pT_o = exp_kt(slotA)
                    avq.append((i, c, 2 * t, pT_e))
                    avq.append((i, c, 2 * t + 1, pT_o))
                    # outproj of chunk 0, spread over c1 heads 1-3 (head 0
                    # is skipped so head 3's chunk-0 normalization chain has
                    # a full window to retire before the first outproj).
                    if c == 1 and i >= 1 and t % 3 == 1:
                        st = (i - 1) * 3 + t // 3
                        if st < 8:
                            while any(e[1] == 0 for e in avq):
                                pop_av()
                            outproj_st(st, tail=False)
        while avq:
            pop_av()

        for st in range(8, NJ):
            outproj_st(st, tail=True)


def build_nc():
    nc = bacc.Bacc("TRN2", target_bir_lowering=False, debug=False, num_devices=8)
    xT = nc.declare_dram_parameter("xT", [ET, P, NH * P], BF16, isOutput=False)
    wqkvT = nc.declare_dram_parameter("wqkvT", [ET, P, 3 * E], BF16, isOutput=False)
    woutT = nc.declare_dram_parameter("woutT", [2, P, E], BF16, isOutput=False)
    bblk = nc.declare_dram_parameter("bblk", [P, 24], F32, isOutput=False)
    outp = nc.declare_dram_parameter("out_part", [S, E], BF16, isOutput=True)
    with tile.TileContext(nc) as tc:
        _emit(nc, tc, xT, wqkvT, woutT, bblk, outp)
    nc.compile()
    return nc


def make_in_maps(x, W_qkv, b_qkv, W_out):
    import ml_dtypes
    bf16 = ml_dtypes.bfloat16
    x = np.asarray(x, np.float32)
    # [ET, P, 3E]: wqkvT[et, p, f] = W_qkv.T[et*128+p, f], cast to bf16
    wqkvT = np.ascontiguousarray(
        np.asarray(W_qkv, np.float32).T.reshape(ET, P, 3 * E)
    ).astype(bf16)
    woutT = np.ascontiguousarray(np.asarray(W_out, np.float32).T)
    b_qkv = np.asarray(b_qkv, np.float32)
    bblk = np.ascontiguousarray(np.asarray(b_qkv, np.float32).reshape(24, P).T)
    in_maps = []
    for core in range(8):
        b, g = divmod(core, 4)
        in_maps.append({
            "xT": np.ascontiguousarray(
                x[b, 512 * g:512 * (g + 1), :].T.reshape(ET, P, NH * P)
            ).astype(bf16),
            "wqkvT": wqkvT,
            "woutT": np.ascontiguousarray(
                woutT[256 * g:256 * (g + 1), :].reshape(2, P, E)
            ).astype(bf16),
            "bblk": bblk,
        })
    return in_maps


def kernel(x, W_qkv, b_qkv, W_out, b_out):
    global _NC_CACHE, _LAST_RESULT
    if _NC_CACHE is None:
        _NC_CACHE = build_nc()
    in_maps = make_in_maps(x, W_qkv, b_qkv, W_out)
    _LAST_RESULT = run_bass_kernel_spmd(_NC_CACHE, in_maps, list(range(8)))
    res = _LAST_RESULT.results
    b_out = np.asarray(b_out, np.float32)
    out = np.empty((B, S, E), np.float32)
    for b in range(B):
        acc = np.asarray(res[4 * b]["out_part"], np.float32).copy()
        for g in range(1, 4):
            acc += np.asarray(res[4 * b + g]["out_part"], np.float32)
        out[b] = acc + b_out
    return out
